# revision 1
# baseline (speedup 1.0000x reference)
"""Causal multi-head attention (B=1, S=4096, D=768, H=12, d_head=64) on 8
Trainium2 NeuronCores.

Sharding: tensor-parallel over heads. 12 heads are mapped onto 16 head-slots
(2 per core); the 4 leftover heads are duplicated onto two slots of the same
core with their W_out rows pre-scaled by 0.5, keeping the SPMD program
uniform across cores. Each core computes Q/K/V projections for its 2 head
slots, causal flash-attention (exp without max-subtraction; softmax
denominator obtained free via an appended ones-column on V), and a partial
row-parallel out-projection. The host sums the 8 partial outputs and adds
b_out (the all-reduce step of the row-parallel out projection).

All matmuls run in float32r with K=128/M=128 (zero-padded where the logical
dims are 64/65) — f32r only hits 1 cycle/row on full 128-wide operands.
"""

import sys

sys.path.insert(0, "/opt/trn_rl_repo")

import numpy as np

import concourse.bass as bass
import concourse.tile as tile
from concourse import bacc, mybir
from concourse.bass_utils import run_bass_kernel_spmd

S = 4096
D = 768
HD = 64
P = 128
KC = D // P  # 6 contraction chunks for the projections
QT_W = 512  # query-tile width (psum free dim)
NQT = S // QT_W  # 8 query tiles
NKB = S // P  # 32 key blocks
NEG = -1e30

F32 = mybir.dt.float32
F32R = mybir.dt.float32r
AF = mybir.ActivationFunctionType

SLOTS = [(0, 1), (2, 3), (4, 5), (6, 7), (8, 8), (9, 9), (10, 10), (11, 11)]
SCALES = [(1.0, 1.0)] * 4 + [(0.5, 0.5)] * 4

_CACHED_NC = None


def build_nc():
    nc = bacc.Bacc("TRN2", target_bir_lowering=False, debug=False, num_devices=8)

    x_d = nc.declare_dram_parameter("x", [S, D], F32, isOutput=False)
    wq_d = nc.declare_dram_parameter("wq", [D, P], F32, isOutput=False)
    wk_d = nc.declare_dram_parameter("wk", [D, P], F32, isOutput=False)
    wv_d = nc.declare_dram_parameter("wv", [D, P], F32, isOutput=False)
    wo_d = nc.declare_dram_parameter("wo", [P, D], F32, isOutput=False)
    mask_d = nc.declare_dram_parameter("mask", [P, P], F32, isOutput=False)
    ident_d = nc.declare_dram_parameter("ident", [P, P], F32, isOutput=False)
    out_d = nc.declare_dram_parameter("out", [S, D], F32, isOutput=True)

    with tile.TileContext(nc) as tc:
        with (
            tc.tile_pool(name="const", bufs=1) as const,
            tc.tile_pool(name="big", bufs=1) as big,
        ):
            # ---- constants ----
            mask_s = const.tile([P, P], F32)
            nc.sync.dma_start(mask_s[:], mask_d[:])
            ident = const.tile([P, P], F32)
            nc.sync.dma_start(ident[:], ident_d[:])
            ident_r = const.tile([P, P], F32R)
            nc.vector.tensor_copy(ident_r[:], ident[:])
            wpool = const  # warmup matmuls: get the PE HAM to 2.4 GHz while
            # the x DMA streams in
            ones_c = const.tile([P, 1], F32)
            nc.gpsimd.memset(ones_c[:], 1.0)
            zero_c = const.tile([P, 1], F32)
            nc.gpsimd.memset(zero_c[:], 0.0)
            wo_r = const.tile([P, D], F32R)

            # qT: slot A rows 0:64, slot B rows 64:128 (no padding needed on
            # the rhs side of the scores matmul). kT per slot, zero-padded on
            # the other 64 rows so the K=128 contraction only picks up its
            # slot. vA: V natural +ones column at 64, zero cols 65:128/slot.
            qT = big.tile([P, S], F32R)
            k2 = [big.tile([P, S], F32R, name=f"k2_{i}") for i in (0, 1)]
            vA = big.tile([P, NKB, 2 * P], F32R)

            nc.vector.tensor_copy(
                k2[0][64:P, :], zero_c[0:64, 0:1].broadcast_to([64, S])
            )
            nc.vector.tensor_copy(
                k2[1][0:64, :], zero_c[0:64, 0:1].broadcast_to([64, S])
            )
            for slot in (0, 1):
                nc.vector.tensor_copy(
                    vA[:, :, slot * P + 65 : slot * P + P],
                    zero_c[:, 0:1].broadcast_to([P, NKB, 63]),
                )
                nc.vector.tensor_copy(
                    vA[:, :, slot * P + 64],
                    ones_c[:, 0:1].broadcast_to([P, NKB]),
                )

            with (
                tc.tile_pool(name="xtp", bufs=1) as xtp,
                tc.tile_pool(name="psA", bufs=2, space="PSUM") as psA,
                tc.tile_pool(name="psB", bufs=4, space="PSUM") as psB,
            ):
                for wi in range(48):
                    wps = psA.tile([P, P], F32, name="tp", tag="tp")
                    nc.tensor.matmul(
                        wps[:], ident_r[:], ident_r[:], start=True, stop=True
                    )
                w_r = xtp.tile([P, KC, 3 * P], F32R)
                with tc.tile_pool(name="wst", bufs=1) as wst:
                    w_stage = wst.tile([P, KC, 3 * P], F32)
                    nc.sync.dma_start(
                        w_stage[:, :, 0:P], wq_d.rearrange("(c p) m -> p c m", p=P)
                    )
                    nc.sync.dma_start(
                        w_stage[:, :, P : 2 * P],
                        wk_d.rearrange("(c p) m -> p c m", p=P),
                    )
                    nc.sync.dma_start(
                        w_stage[:, :, 2 * P : 3 * P],
                        wv_d.rearrange("(c p) m -> p c m", p=P),
                    )
                    nc.vector.tensor_copy(w_r[:], w_stage[:])
                    wo_stage = wst.tile([P, D], F32)
                    nc.sync.dma_start(wo_stage[:], wo_d[:])
                    nc.vector.tensor_copy(wo_r[:], wo_stage[:])

                # ---- phases 1+2 interleaved: per q-tile group, DMA x,
                # transpose via PE, then Q/K/V projections for that group ----
                xT = xtp.tile([P, KC, S], F32R)
                with tc.tile_pool(name="xs", bufs=4) as xs:
                    for t in range(NQT):
                        for sti in range(4):
                            st = t * 4 + sti
                            for half in range(2):
                                x_stage = xs.tile([P, D // 2], F32)
                                nc.sync.dma_start(
                                    x_stage[:],
                                    x_d[
                                        st * P : (st + 1) * P,
                                        half * (D // 2) : (half + 1) * (D // 2),
                                    ],
                                )
                                for ci in range(KC // 2):
                                    c = half * (KC // 2) + ci
                                    tp = psA.tile([P, P], F32)
                                    nc.tensor.transpose(
                                        tp[:],
                                        x_stage[:, ci * P : (ci + 1) * P],
                                        ident[:],
                                    )
                                    nc.vector.tensor_copy(
                                        xT[:, c, st * P : (st + 1) * P], tp[:]
                                    )
                        # Q projection for this q-tile group
                        pj = psB.tile([P, QT_W], F32, name="pjq", tag="pj")
                        for c in range(KC):
                            nc.tensor.matmul(
                                pj[:],
                                w_r[:, c, 0:P],
                                xT[:, c, t * QT_W : (t + 1) * QT_W],
                                start=(c == 0),
                                stop=(c == KC - 1),
                            )
                        nc.vector.tensor_copy(qT[:, t * QT_W : (t + 1) * QT_W], pj[:])
                        # K projection
                        pj = psB.tile([P, QT_W], F32, name="pjk", tag="pj")
                        for c in range(KC):
                            nc.tensor.matmul(
                                pj[:],
                                w_r[:, c, P : 2 * P],
                                xT[:, c, t * QT_W : (t + 1) * QT_W],
                                start=(c == 0),
                                stop=(c == KC - 1),
                            )
                        nc.vector.tensor_copy(
                            k2[0][0:64, t * QT_W : (t + 1) * QT_W], pj[0:64, :]
                        )
                        nc.vector.tensor_copy(
                            k2[1][64:P, t * QT_W : (t + 1) * QT_W], pj[64:P, :]
                        )
                        # V projection + transpose to natural layout
                        pj = psB.tile([P, QT_W], F32, name="pjv", tag="pj")
                        for c in range(KC):
                            nc.tensor.matmul(
                                pj[:],
                                w_r[:, c, 2 * P : 3 * P],
                                xT[:, c, t * QT_W : (t + 1) * QT_W],
                                start=(c == 0),
                                stop=(c == KC - 1),
                            )
                        vt_t = xtp.tile(
                            [P, QT_W], F32R, name="vt_t", tag="vt_t", bufs=2
                        )
                        nc.vector.tensor_copy(vt_t[:], pj[:])
                        for b in range(QT_W // P):
                            kb = t * 4 + b
                            tp2 = psA.tile([P, P], F32R)
                            nc.tensor.transpose(
                                tp2[:], vt_t[:, b * P : (b + 1) * P], ident_r[:]
                            )
                            nc.vector.tensor_copy(vA[:, kb, 0:64], tp2[:, 0:64])
                            nc.vector.tensor_copy(
                                vA[:, kb, P : P + 64], tp2[:, 64:P]
                            )

            # ---- phase 3: attention ----
            cT = None
            with tc.tile_pool(name="ctx_sb", bufs=1) as ctx_sb:
              cT = ctx_sb.tile([P, S], F32R)
              with (
                tc.tile_pool(name="scp", bufs=4, space="PSUM") as scp,
                tc.tile_pool(name="ctp", bufs=2, space="PSUM") as ctp,
                tc.tile_pool(name="pt", bufs=8) as pt,
                tc.tile_pool(name="sm", bufs=4) as sm,
              ):
                def outproj(st):
                    o_stage = sm.tile([P, D], F32, name="o_stage", bufs=3)
                    for nch in range(2):
                        po = scp.tile([P, QT_W], F32, name="sc", tag="sc")
                        nc.tensor.matmul(
                            po[:, : D // 2],
                            cT[:, st * P : (st + 1) * P],
                            wo_r[:, nch * (D // 2) : (nch + 1) * (D // 2)],
                            start=True,
                            stop=True,
                        )
                        nc.vector.tensor_copy(
                            o_stage[:, nch * (D // 2) : (nch + 1) * (D // 2)],
                            po[:, : D // 2],
                        )
                    nc.sync.dma_start(out_d[st * P : (st + 1) * P, :], o_stage[:])

                for t in range(NQT):
                    if t == NQT - 1:
                        # rows covered by tiles 0-6 are final; overlap their
                        # out-projection with the last (largest) q-tile
                        for st in range(28):
                            outproj(st)
                    nkb = 4 * (t + 1)
                    ctx_ps = [
                        ctp.tile([P, QT_W], F32, name=f"ctx{s}", tag=f"ctx{s}")
                        for s in (0, 1)
                    ]
                    for kb in range(nkb):
                        r = kb * P - t * QT_W  # diagonal offset
                        r0 = max(0, r)
                        p_tiles = []
                        for slot in (0, 1):
                            sc = scp.tile([P, QT_W], F32, name="sc", tag="sc")
                            nc.tensor.matmul(
                                sc[:],
                                k2[slot][:, kb * P : (kb + 1) * P],
                                qT[:, t * QT_W : (t + 1) * QT_W],
                                start=True,
                                stop=True,
                            )
                            if r >= 0:
                                nc.vector.tensor_tensor(
                                    sc[:, r : r + P],
                                    sc[:, r : r + P],
                                    mask_s[:],
                                    mybir.AluOpType.add,
                                )
                            p_t = pt.tile([P, QT_W], F32R, name="ptile")
                            nc.scalar.activation(
                                p_t[:, r0:QT_W],
                                sc[:, r0:QT_W],
                                AF.Exp,
                                scale=0.125,
                            )
                            p_tiles.append(p_t)
                        for slot in (0, 1):
                            nc.tensor.matmul(
                                ctx_ps[slot][:, r0:QT_W],
                                vA[:, kb, slot * P : (slot + 1) * P],
                                p_tiles[slot][:, r0:QT_W],
                                start=(kb == 0),
                                stop=(kb == nkb - 1),
                            )
                    for slot in (0, 1):
                        lr = sm.tile([1, QT_W], F32, name="lrecip")
                        nc.vector.reciprocal(lr[:], ctx_ps[slot][64:65, :])
                        lb = sm.tile([64, QT_W], F32, name="lb")
                        nc.gpsimd.partition_broadcast(lb[:], lr[0:1, :])
                        nc.vector.tensor_tensor(
                            cT[slot * 64 : slot * 64 + 64, t * QT_W : (t + 1) * QT_W],
                            ctx_ps[slot][0:64, :],
                            lb[:],
                            mybir.AluOpType.mult,
                        )

                for st in range(28, S // P):
                    outproj(st)



    nc.compile()
    return nc


def _host_inputs(x, W_query, W_key, W_value, W_out):
    mask = np.where(
        np.arange(P)[:, None] <= np.arange(P)[None, :], 0.0, NEG
    ).astype(np.float32)
    ident = np.eye(P, dtype=np.float32)
    in_maps = []
    for core in range(8):
        ha, hb = SLOTS[core]
        sa, sb = SCALES[core]
        ca, cb = slice(ha * HD, (ha + 1) * HD), slice(hb * HD, (hb + 1) * HD)
        in_maps.append(
            {
                "x": np.ascontiguousarray(x),
                "wq": np.ascontiguousarray(
                    np.concatenate([W_query[:, ca], W_query[:, cb]], axis=1)
                ),
                "wk": np.ascontiguousarray(
                    np.concatenate([W_key[:, ca], W_key[:, cb]], axis=1)
                ),
                "wv": np.ascontiguousarray(
                    np.concatenate([W_value[:, ca], W_value[:, cb]], axis=1)
                ),
                "wo": np.ascontiguousarray(
                    np.concatenate([W_out[ca, :] * sa, W_out[cb, :] * sb], axis=0)
                ),
                "mask": mask,
                "ident": ident,
            }
        )
    return in_maps


def run(x, W_query, W_key, W_value, W_out, b_out, trace=False):
    global _CACHED_NC
    if _CACHED_NC is None:
        _CACHED_NC = build_nc()
    nc = _CACHED_NC
    in_maps = _host_inputs(x, W_query, W_key, W_value, W_out)
    res = run_bass_kernel_spmd(nc, in_maps, core_ids=list(range(8)), trace=trace)
    out = np.zeros((S, D), dtype=np.float32)
    for core in range(8):
        out += res.results[core]["out"]
    out += b_out[None, :].astype(np.float32)
    return out, res


def kernel(x, W_query, W_key, W_value, W_out, b_out):
    x2 = np.asarray(x, dtype=np.float32).reshape(S, D)
    out, _ = run(
        x2,
        np.asarray(W_query, np.float32),
        np.asarray(W_key, np.float32),
        np.asarray(W_value, np.float32),
        np.asarray(W_out, np.float32),
        np.asarray(b_out, np.float32),
    )
    return out.reshape(1, S, D)



# revision 22
# speedup vs baseline: 1.2436x; 1.2436x over previous
"""Causal multi-head attention (B=1, S=4096, D=768, H=12, d_head=64) on 8
Trainium2 NeuronCores.

Sharding: tensor-parallel over heads. 12 heads are mapped onto 16 head-slots
(2 per core); the 4 leftover heads are duplicated onto two slots of the same
core with their W_out rows pre-scaled by 0.5, keeping the SPMD program
uniform across cores. The host sums the 8 partial row-parallel
out-projection outputs and adds b_out.

All matmul operands are bf16 (PSUM accumulates f32): x arrives
host-pre-transposed as xT [768, 4096] bf16 so no on-device transposes or
casts are needed to feed the Q/K/V projections. Q/K/V for one query tile
share a single 3-bank PSUM tile. exp runs on 3-block groups
(ACT instruction overhead amortized) with scale=1/8 and bias=-40 (cancels in
normalization, keeps unnormalized weights in range). Softmax denominators
come free as a ones-column appended to V in the PV stationary; their
reciprocal uses the fast approx DVE op. The out-projection streams per query
tile and the partial output is written in bf16 (halves output DMA).
"""

import sys

sys.path.insert(0, "/opt/trn_rl_repo")

import ml_dtypes
import numpy as np

import concourse.bass as bass
import concourse.tile as tile
from concourse import bacc, mybir
from concourse.bass_utils import run_bass_kernel_spmd

S = 4096
D = 768
HD = 64
P = 128
KC = D // P  # 6 contraction chunks for the projections
QT_W = 512  # query-tile width (one psum bank of f32)
NQT = S // QT_W  # 8 query tiles
NKB = S // P  # 32 key blocks
GRP = 3  # score blocks per exp group (3 psum banks)

F32 = mybir.dt.float32
BF16 = mybir.dt.bfloat16
AF = mybir.ActivationFunctionType
EXP_BIAS = -40.0

SLOTS = [(0, 1), (2, 3), (4, 5), (6, 7), (8, 8), (9, 9), (10, 10), (11, 11)]
SCALES = [(1.0, 1.0)] * 4 + [(0.5, 0.5)] * 4

DEBUG = False  # add DRAM dumps of intermediates (qT/k2/vA/cT)
_CACHED_NC = None


def build_nc():
    nc = bacc.Bacc("TRN2", target_bir_lowering=False, debug=False, num_devices=8)

    xt_d = nc.declare_dram_parameter("xt", [D, S], BF16, isOutput=False)
    wq_d = nc.declare_dram_parameter("wq", [D, P], BF16, isOutput=False)
    wk_d = nc.declare_dram_parameter("wk", [D, P], BF16, isOutput=False)
    wv_d = nc.declare_dram_parameter("wv", [D, P], BF16, isOutput=False)
    wo_d = nc.declare_dram_parameter("wo", [P, D], BF16, isOutput=False)
    mask_d = nc.declare_dram_parameter("mask", [P, P], F32, isOutput=False)
    ident_d = nc.declare_dram_parameter("ident", [P, P], BF16, isOutput=False)
    out_d = nc.declare_dram_parameter("out", [S, D], BF16, isOutput=True)
    if DEBUG:
        dbg_x_d = nc.declare_dram_parameter("dbg_x", [P, KC, S], BF16, isOutput=True)
        dbg_q_d = nc.declare_dram_parameter("dbg_q", [P, S], BF16, isOutput=True)
        dbg_k0_d = nc.declare_dram_parameter("dbg_k0", [P, S], BF16, isOutput=True)
        dbg_va_d = nc.declare_dram_parameter(
            "dbg_va", [P, NKB, 2 * P], BF16, isOutput=True
        )
        dbg_ct_d = nc.declare_dram_parameter("dbg_ct", [P, S], BF16, isOutput=True)
        dbg_sc_d = nc.declare_dram_parameter("dbg_sc", [P, QT_W], F32, isOutput=True)
        dbg_p_d = nc.declare_dram_parameter("dbg_p", [P, QT_W], BF16, isOutput=True)
        dbg_cx_d = nc.declare_dram_parameter("dbg_cx", [P, QT_W], F32, isOutput=True)

    with tile.TileContext(nc) as tc:
        with (
            tc.tile_pool(name="const", bufs=1) as const,
            tc.tile_pool(name="big", bufs=1) as big,
        ):
            # ---- constants / persistent SBUF ----
            mask_s = const.tile([P, P], F32)
            nc.sync.dma_start(mask_s[:], mask_d[:])
            ident = const.tile([P, P], BF16)
            nc.sync.dma_start(ident[:], ident_d[:])
            w_r = const.tile([P, KC, 3 * P], BF16)
            nc.sync.dma_start(w_r[:, :, 0:P], wq_d.rearrange("(c p) m -> p c m", p=P))
            nc.sync.dma_start(
                w_r[:, :, P : 2 * P], wk_d.rearrange("(c p) m -> p c m", p=P)
            )
            nc.sync.dma_start(
                w_r[:, :, 2 * P : 3 * P], wv_d.rearrange("(c p) m -> p c m", p=P)
            )
            wo_r = const.tile([P, D], BF16)
            nc.sync.dma_start(wo_r[:], wo_d[:])

            warm = const.tile([P, QT_W], BF16)
            nc.gpsimd.memset(warm[:], 0.5)
            ebias = const.tile([P, 1], F32)
            nc.gpsimd.memset(ebias[:], EXP_BIAS)

            xT = big.tile([P, KC, S], BF16)  # d-on-partitions x, streamed in
            qT = big.tile([P, S], BF16)  # rows 0:64 slot A, 64:128 slot B
            # zero-padded per-slot keys: partial (64-row / 65-col) stationary
            # tiles run the PE at half rate, so keep every stationary 128x128
            k2 = [big.tile([P, S], BF16, name=f"k2_{i}") for i in (0, 1)]
            # vA[key, kb, slot*128+j]: j 0 = ones (denominator lands on psum
            # partition 0, where the custom recip op needs it; psum partition
            # bases must be 32-aligned so ctx values go to partitions 64:128),
            # j 1:64 = 0, j 64:128 = V_slot
            vA = big.tile([P, NKB, 2 * P], BF16)
            cT = big.tile([P, S], BF16)  # normalized ctx: 0:64 A, 64:128 B

            nc.gpsimd.memset(k2[0][HD:P, :], 0.0)
            nc.gpsimd.memset(k2[1][0:HD, :], 0.0)
            for slot in (0, 1):
                nc.gpsimd.memset(vA[:, :, slot * P], 1.0)
                nc.gpsimd.memset(vA[:, :, slot * P + 1 : slot * P + HD], 0.0)

            # ---- phase 1: stream x, project Q/K/V, build vA ----
            with (
                tc.tile_pool(name="pjp", bufs=2, space="PSUM") as pjp,
                tc.tile_pool(name="vtp", bufs=2, space="PSUM") as vtp,
                tc.tile_pool(name="stg", bufs=2) as stg,
            ):
                # warm the PE p-state while the first x chunk streams in
                for wi in range(16):
                    wps = pjp.tile([P, 3 * QT_W], F32, name="pj", tag="pj")
                    nc.tensor.matmul(
                        wps[:, 0:QT_W], ident[:], warm[:], start=True, stop=True
                    )
                for t in range(NQT):
                    nc.sync.dma_start(
                        xT[:, :, t * QT_W : (t + 1) * QT_W],
                        xt_d.rearrange("(c p) s -> p c s", p=P)[
                            :, :, t * QT_W : (t + 1) * QT_W
                        ],
                    )
                    pj = pjp.tile([P, 3 * QT_W], F32, name="pj", tag="pj")
                    for j in range(3):  # Q, K, V share one 3-bank psum tile
                        for c in range(KC):
                            nc.tensor.matmul(
                                pj[:, j * QT_W : (j + 1) * QT_W],
                                w_r[:, c, j * P : (j + 1) * P],
                                xT[:, c, t * QT_W : (t + 1) * QT_W],
                                start=(c == 0),
                                stop=(c == KC - 1),
                            )
                    nc.vector.tensor_copy(
                        qT[:, t * QT_W : (t + 1) * QT_W], pj[:, 0:QT_W]
                    )
                    nc.vector.tensor_copy(
                        k2[0][0:HD, t * QT_W : (t + 1) * QT_W],
                        pj[0:HD, QT_W : 2 * QT_W],
                    )
                    nc.vector.tensor_copy(
                        k2[1][HD:P, t * QT_W : (t + 1) * QT_W],
                        pj[HD:P, QT_W : 2 * QT_W],
                    )
                    vt = stg.tile([P, QT_W], BF16, name="vt", tag="vt")
                    nc.vector.tensor_copy(vt[:], pj[:, 2 * QT_W : 3 * QT_W])
                    for b in range(QT_W // P):
                        kb = t * 4 + b
                        tp = vtp.tile([P, P], BF16, name="tp", tag="tp")
                        nc.tensor.transpose(
                            tp[:], vt[:, b * P : (b + 1) * P], ident[:]
                        )
                        nc.vector.tensor_copy(vA[:, kb, HD:P], tp[:, 0:HD])
                        nc.vector.tensor_copy(vA[:, kb, P + HD : 2 * P], tp[:, HD:P])

            # ---- phase 2: attention + out-projection ----
            with (
                tc.tile_pool(name="scp", bufs=2, space="PSUM") as scp,
                tc.tile_pool(name="ctp", bufs=2, space="PSUM") as ctp,
                tc.tile_pool(name="pt", bufs=3) as pt,
                tc.tile_pool(name="sm", bufs=3) as sm,
            ):

                def attend(t, slot):
                    """Head slot 0/1: qdims at rows [slot*64, slot*64+64)."""
                    off = slot * HD
                    nkb = 4 * (t + 1)
                    ctx = ctp.tile([P, QT_W], F32, name="ctx", tag="ctx")
                    q_mv = qT[:, t * QT_W : (t + 1) * QT_W]
                    for g0 in range(0, nkb, GRP):
                        kbs = range(g0, min(g0 + GRP, nkb))
                        gw = len(kbs) * QT_W
                        sc = scp.tile([P, GRP * QT_W], F32, name="sc", tag="sc")
                        for i, kb in enumerate(kbs):
                            nc.tensor.matmul(
                                sc[:, i * QT_W : (i + 1) * QT_W],
                                k2[slot][:, kb * P : (kb + 1) * P],
                                q_mv,
                                start=True,
                                stop=True,
                            )
                        for i, kb in enumerate(kbs):
                            r = kb * P - t * QT_W
                            if r >= 0:
                                nc.vector.tensor_tensor(
                                    sc[:, i * QT_W + r : i * QT_W + r + P],
                                    sc[:, i * QT_W + r : i * QT_W + r + P],
                                    mask_s[:],
                                    mybir.AluOpType.add,
                                )
                        if DEBUG and t == 0 and slot == 0 and g0 == 0:
                            dsc = sm.tile([P, QT_W], F32, name="dsc", tag="dsc")
                            nc.vector.tensor_copy(dsc[:], sc[:, 0:QT_W])
                            nc.sync.dma_start(dbg_sc_d[:], dsc[:])
                        p_t = pt.tile([P, GRP * QT_W], BF16, name="ptile")
                        nc.scalar.activation(
                            p_t[:, 0:gw],
                            sc[:, 0:gw],
                            AF.Exp,
                            scale=0.125,
                            bias=ebias[:],
                        )
                        if DEBUG and t == 0 and slot == 0 and g0 == 0:
                            nc.sync.dma_start(dbg_p_d[:], p_t[:, 0:QT_W])
                        for i, kb in enumerate(kbs):
                            r0 = max(0, kb * P - t * QT_W)
                            nc.tensor.matmul(
                                ctx[:, r0:QT_W],
                                vA[:, kb, slot * P : (slot + 1) * P],
                                p_t[:, i * QT_W + r0 : (i + 1) * QT_W],
                                start=(kb == 0),
                                stop=(kb == nkb - 1),
                            )
                    if DEBUG and t == 0 and slot == 0:
                        dcx = sm.tile([P, QT_W], F32, name="dcx", tag="dcx")
                        nc.vector.tensor_copy(dcx[:], ctx[:])
                        nc.sync.dma_start(dbg_cx_d[:], dcx[:])
                    rr = sm.tile([1, QT_W], F32, name="rr", tag="rr")
                    nc.vector.reciprocal_approx_fast(rr[:], ctx[0:1, :])
                    lb = sm.tile([HD, QT_W], F32, name="lb", tag="lb")
                    nc.gpsimd.partition_broadcast(lb[:], rr[0:1, :])
                    nc.vector.tensor_tensor(
                        cT[off : off + HD, t * QT_W : (t + 1) * QT_W],
                        ctx[HD:P, :],
                        lb[:],
                        mybir.AluOpType.mult,
                    )

                def outproj(t):
                    for b in range(QT_W // P):
                        st = t * 4 + b
                        po = scp.tile([P, GRP * QT_W], F32, name="sc", tag="sc")
                        o_stage = sm.tile([P, D], BF16, name="o_stage", tag="ost")
                        for nch in range(2):
                            # psum offsets 0 and 512 keep each 384-wide half
                            # inside a single 2KB bank
                            pof = nch * QT_W
                            nc.tensor.matmul(
                                po[:, pof : pof + D // 2],
                                cT[:, st * P : (st + 1) * P],
                                wo_r[:, nch * (D // 2) : (nch + 1) * (D // 2)],
                                start=True,
                                stop=True,
                            )
                            nc.vector.tensor_copy(
                                o_stage[:, nch * (D // 2) : (nch + 1) * (D // 2)],
                                po[:, pof : pof + D // 2],
                            )
                        nc.sync.dma_start(out_d[st * P : (st + 1) * P, :], o_stage[:])

                for t in range(NQT):
                    attend(t, 0)
                    attend(t, 1)
                    outproj(t)

                if DEBUG:
                    nc.sync.dma_start(dbg_x_d[:], xT[:])
                    nc.sync.dma_start(dbg_q_d[:], qT[:])
                    nc.sync.dma_start(dbg_k0_d[:], k2[0][:])
                    nc.sync.dma_start(dbg_va_d[:], vA[:])
                    nc.sync.dma_start(dbg_ct_d[:], cT[:])

    nc.compile()
    return nc


def _host_inputs(x, W_query, W_key, W_value, W_out):
    mask = np.where(
        np.arange(P)[:, None] <= np.arange(P)[None, :], 0.0, -1e30
    ).astype(np.float32)
    ident = np.eye(P, dtype=ml_dtypes.bfloat16)
    xt = np.ascontiguousarray(x.T).astype(ml_dtypes.bfloat16)
    in_maps = []
    for core in range(8):
        ha, hb = SLOTS[core]
        sa, sb = SCALES[core]
        ca, cb = slice(ha * HD, (ha + 1) * HD), slice(hb * HD, (hb + 1) * HD)
        in_maps.append(
            {
                "xt": xt,
                "wq": np.ascontiguousarray(
                    np.concatenate([W_query[:, ca], W_query[:, cb]], axis=1)
                ).astype(ml_dtypes.bfloat16),
                "wk": np.ascontiguousarray(
                    np.concatenate([W_key[:, ca], W_key[:, cb]], axis=1)
                ).astype(ml_dtypes.bfloat16),
                "wv": np.ascontiguousarray(
                    np.concatenate([W_value[:, ca], W_value[:, cb]], axis=1)
                ).astype(ml_dtypes.bfloat16),
                "wo": np.ascontiguousarray(
                    np.concatenate([W_out[ca, :] * sa, W_out[cb, :] * sb], axis=0)
                ).astype(ml_dtypes.bfloat16),
                "mask": mask,
                "ident": ident,
            }
        )
    return in_maps


def run(x, W_query, W_key, W_value, W_out, b_out, trace=False):
    global _CACHED_NC
    if _CACHED_NC is None:
        _CACHED_NC = build_nc()
    nc = _CACHED_NC
    in_maps = _host_inputs(x, W_query, W_key, W_value, W_out)
    res = run_bass_kernel_spmd(nc, in_maps, core_ids=list(range(8)), trace=trace)
    out = np.zeros((S, D), dtype=np.float32)
    for core in range(8):
        out += res.results[core]["out"].astype(np.float32)
    out += b_out[None, :].astype(np.float32)
    return out, res


def kernel(x, W_query, W_key, W_value, W_out, b_out):
    x2 = np.asarray(x, dtype=np.float32).reshape(S, D)
    out, _ = run(
        x2,
        np.asarray(W_query, np.float32),
        np.asarray(W_key, np.float32),
        np.asarray(W_value, np.float32),
        np.asarray(W_out, np.float32),
        np.asarray(b_out, np.float32),
    )
    return out.reshape(1, S, D)


# revision 28
# speedup vs baseline: 1.5944x; 1.2821x over previous
"""Causal multi-head attention (B=1, S=4096, D=768, H=12, d_head=64) on 8
Trainium2 NeuronCores.

Sharding: tensor-parallel over heads. 12 heads are mapped onto 16 head-slots
(2 per core); the 4 leftover heads are duplicated onto two slots of the same
core with their W_out rows pre-scaled by 0.5, keeping the SPMD program
uniform across cores. The host sums the 8 partial row-parallel
out-projection outputs and adds b_out.

All matmul operands are bf16 (PSUM accumulates f32): x arrives
host-pre-transposed as xT [768, 4096] bf16 so no on-device transposes or
casts are needed to feed the Q/K/V projections. Q/K/V for one query tile
share a single 3-bank PSUM tile. exp runs on 3-block groups
(ACT instruction overhead amortized) with scale=1/8 and bias=-40 (cancels in
normalization, keeps unnormalized weights in range). Softmax denominators
come free as a ones-column appended to V in the PV stationary; their
reciprocal uses the fast approx DVE op. The out-projection streams per query
tile and the partial output is written in bf16 (halves output DMA).
"""

import sys

sys.path.insert(0, "/opt/trn_rl_repo")

import ml_dtypes
import numpy as np

import concourse.bass as bass
import concourse.tile as tile
from concourse import bacc, mybir
from concourse.bass_utils import run_bass_kernel_spmd

S = 4096
D = 768
HD = 64
P = 128
KC = D // P  # 6 contraction chunks for the projections
QT_W = 512  # query-tile width (one psum bank of f32)
NQT = S // QT_W  # 8 query tiles
NKB = S // P  # 32 key blocks
GRP = 3  # score blocks per exp group (3 psum banks)

F32 = mybir.dt.float32
BF16 = mybir.dt.bfloat16
AF = mybir.ActivationFunctionType
EXP_BIAS = -40.0

SLOTS = [(0, 1), (2, 3), (4, 5), (6, 7), (8, 8), (9, 9), (10, 10), (11, 11)]
SCALES = [(1.0, 1.0)] * 4 + [(0.5, 0.5)] * 4

DEBUG = False  # add DRAM dumps of intermediates (qT/k2/vA/cT)
_CACHED_NC = None


def build_nc():
    nc = bacc.Bacc("TRN2", target_bir_lowering=False, debug=False, num_devices=8)

    xt_d = nc.declare_dram_parameter("xt", [D, S], BF16, isOutput=False)
    wq_d = nc.declare_dram_parameter("wq", [D, P], BF16, isOutput=False)
    wk_d = nc.declare_dram_parameter("wk", [D, P], BF16, isOutput=False)
    wv_d = nc.declare_dram_parameter("wv", [D, P], BF16, isOutput=False)
    wo_d = nc.declare_dram_parameter("wo", [P, D], BF16, isOutput=False)
    mask_d = nc.declare_dram_parameter("mask", [P, P], F32, isOutput=False)
    ident_d = nc.declare_dram_parameter("ident", [P, P], BF16, isOutput=False)
    out_d = nc.declare_dram_parameter("out", [S, D], BF16, isOutput=True)
    if DEBUG:
        dbg_x_d = nc.declare_dram_parameter("dbg_x", [P, KC, S], BF16, isOutput=True)
        dbg_q_d = nc.declare_dram_parameter("dbg_q", [P, S], BF16, isOutput=True)
        dbg_k0_d = nc.declare_dram_parameter("dbg_k0", [P, S], BF16, isOutput=True)
        dbg_va_d = nc.declare_dram_parameter(
            "dbg_va", [P, NKB, 2 * P], BF16, isOutput=True
        )
        dbg_ct_d = nc.declare_dram_parameter("dbg_ct", [P, S], BF16, isOutput=True)
        dbg_sc_d = nc.declare_dram_parameter("dbg_sc", [P, QT_W], F32, isOutput=True)
        dbg_p_d = nc.declare_dram_parameter("dbg_p", [P, QT_W], BF16, isOutput=True)
        dbg_cx_d = nc.declare_dram_parameter("dbg_cx", [P, QT_W], F32, isOutput=True)

    with tile.TileContext(nc) as tc:
        with (
            tc.tile_pool(name="const", bufs=1) as const,
            tc.tile_pool(name="big", bufs=1) as big,
        ):
            # ---- constants / persistent SBUF ----
            mask_s = const.tile([P, P], F32)
            nc.sync.dma_start(mask_s[:], mask_d[:])
            ident = const.tile([P, P], BF16)
            nc.sync.dma_start(ident[:], ident_d[:])
            w_r = const.tile([P, KC, 3 * P], BF16)
            nc.sync.dma_start(w_r[:, :, 0:P], wq_d.rearrange("(c p) m -> p c m", p=P))
            nc.sync.dma_start(
                w_r[:, :, P : 2 * P], wk_d.rearrange("(c p) m -> p c m", p=P)
            )
            nc.sync.dma_start(
                w_r[:, :, 2 * P : 3 * P], wv_d.rearrange("(c p) m -> p c m", p=P)
            )
            wo_r = const.tile([P, D], BF16)
            nc.sync.dma_start(wo_r[:], wo_d[:])

            warm = const.tile([P, QT_W], BF16)
            nc.gpsimd.memset(warm[:], 0.5)
            ebias = const.tile([P, 1], F32)
            nc.gpsimd.memset(ebias[:], EXP_BIAS)

            xT = big.tile([P, KC, S], BF16)  # d-on-partitions x, streamed in
            qT = big.tile([P, S], BF16)  # rows 0:64 slot A, 64:128 slot B
            # zero-padded per-slot keys: partial (64-row / 65-col) stationary
            # tiles run the PE at half rate, so keep every stationary 128x128
            k2 = [big.tile([P, S], BF16, name=f"k2_{i}") for i in (0, 1)]
            # vA[key, kb, slot*128+j]: j 0 = ones (denominator lands on psum
            # partition 0, where the custom recip op needs it; psum partition
            # bases must be 32-aligned so ctx values go to partitions 64:128),
            # j 1:64 = 0, j 64:128 = V_slot
            vA = big.tile([P, NKB, 2 * P], BF16)
            cT = big.tile([P, S], BF16)  # normalized ctx: 0:64 A, 64:128 B

            nc.gpsimd.memset(k2[0][HD:P, :], 0.0)
            nc.gpsimd.memset(k2[1][0:HD, :], 0.0)
            for slot in (0, 1):
                nc.gpsimd.memset(vA[:, :, slot * P], 1.0)
                nc.gpsimd.memset(vA[:, :, slot * P + 1 : slot * P + HD], 0.0)

            # one psum pool for the whole kernel: tag "sc" = 2 x 3 banks
            # (proj Q/K/V triple, score groups), tag "ctx" = 2 x 1 bank
            # (V-transposes, ctx accumulators, out-proj halves) -> 8 banks,
            # no phase barrier between projection and attention
            with (
                tc.tile_pool(name="scp", bufs=2, space="PSUM") as scp,
                tc.tile_pool(name="stg", bufs=2) as stg,
                tc.tile_pool(name="pt", bufs=3) as pt,
                tc.tile_pool(name="sm", bufs=3) as sm,
            ):
                # warm the PE p-state while the first x chunk streams in
                for wi in range(16):
                    wps = scp.tile([P, GRP * QT_W], F32, name="sc", tag="sc")
                    nc.tensor.matmul(
                        wps[:, 0:QT_W], ident[:], warm[:], start=True, stop=True
                    )
                for t in range(NQT):
                    nc.sync.dma_start(
                        xT[:, :, t * QT_W : (t + 1) * QT_W],
                        xt_d.rearrange("(c p) s -> p c s", p=P)[
                            :, :, t * QT_W : (t + 1) * QT_W
                        ],
                    )
                    pj = scp.tile([P, GRP * QT_W], F32, name="sc", tag="sc")
                    for j in range(3):  # Q, K, V share one 3-bank psum tile
                        for c in range(KC):
                            nc.tensor.matmul(
                                pj[:, j * QT_W : (j + 1) * QT_W],
                                w_r[:, c, j * P : (j + 1) * P],
                                xT[:, c, t * QT_W : (t + 1) * QT_W],
                                start=(c == 0),
                                stop=(c == KC - 1),
                            )
                    nc.vector.tensor_copy(
                        qT[:, t * QT_W : (t + 1) * QT_W], pj[:, 0:QT_W]
                    )
                    nc.vector.tensor_copy(
                        k2[0][0:HD, t * QT_W : (t + 1) * QT_W],
                        pj[0:HD, QT_W : 2 * QT_W],
                    )
                    nc.vector.tensor_copy(
                        k2[1][HD:P, t * QT_W : (t + 1) * QT_W],
                        pj[HD:P, QT_W : 2 * QT_W],
                    )
                    vt = stg.tile([P, QT_W], BF16, name="vt", tag="vt")
                    nc.vector.tensor_copy(vt[:], pj[:, 2 * QT_W : 3 * QT_W])
                    for b in range(QT_W // P):
                        kb = t * 4 + b
                        tp = scp.tile(
                            [P, P], BF16, name="ctx", tag="ctx", padded_shape=None
                        )
                        nc.tensor.transpose(
                            tp[:], vt[:, b * P : (b + 1) * P], ident[:]
                        )
                        nc.vector.tensor_copy(vA[:, kb, HD:P], tp[:, 0:HD])
                        nc.vector.tensor_copy(vA[:, kb, P + HD : 2 * P], tp[:, HD:P])

                # ---- attention + out-projection ----
                def attend(t, slot):
                    """Head slot 0/1: qdims at rows [slot*64, slot*64+64)."""
                    off = slot * HD
                    nkb = 4 * (t + 1)
                    ctx = scp.tile([P, QT_W], F32, name="ctx", tag="ctx")
                    q_mv = qT[:, t * QT_W : (t + 1) * QT_W]
                    for g0 in range(0, nkb, GRP):
                        kbs = range(g0, min(g0 + GRP, nkb))
                        gw = len(kbs) * QT_W
                        sc = scp.tile([P, GRP * QT_W], F32, name="sc", tag="sc")
                        for i, kb in enumerate(kbs):
                            nc.tensor.matmul(
                                sc[:, i * QT_W : (i + 1) * QT_W],
                                k2[slot][:, kb * P : (kb + 1) * P],
                                q_mv,
                                start=True,
                                stop=True,
                            )
                        for i, kb in enumerate(kbs):
                            r = kb * P - t * QT_W
                            if r >= 0:
                                nc.vector.tensor_tensor(
                                    sc[:, i * QT_W + r : i * QT_W + r + P],
                                    sc[:, i * QT_W + r : i * QT_W + r + P],
                                    mask_s[:],
                                    mybir.AluOpType.add,
                                )
                        if DEBUG and t == 0 and slot == 0 and g0 == 0:
                            dsc = sm.tile([P, QT_W], F32, name="dsc", tag="dsc")
                            nc.vector.tensor_copy(dsc[:], sc[:, 0:QT_W])
                            nc.sync.dma_start(dbg_sc_d[:], dsc[:])
                        p_t = pt.tile([P, GRP * QT_W], BF16, name="ptile")
                        nc.scalar.activation(
                            p_t[:, 0:gw],
                            sc[:, 0:gw],
                            AF.Exp,
                            scale=0.125,
                            bias=ebias[:],
                        )
                        if DEBUG and t == 0 and slot == 0 and g0 == 0:
                            nc.sync.dma_start(dbg_p_d[:], p_t[:, 0:QT_W])
                        for i, kb in enumerate(kbs):
                            r0 = max(0, kb * P - t * QT_W)
                            nc.tensor.matmul(
                                ctx[:, r0:QT_W],
                                vA[:, kb, slot * P : (slot + 1) * P],
                                p_t[:, i * QT_W + r0 : (i + 1) * QT_W],
                                start=(kb == 0),
                                stop=(kb == nkb - 1),
                            )
                    if DEBUG and t == 0 and slot == 0:
                        dcx = sm.tile([P, QT_W], F32, name="dcx", tag="dcx")
                        nc.vector.tensor_copy(dcx[:], ctx[:])
                        nc.sync.dma_start(dbg_cx_d[:], dcx[:])
                    rr = sm.tile([1, QT_W], F32, name="rr", tag="rr")
                    nc.vector.reciprocal_approx_fast(rr[:], ctx[0:1, :])
                    lb = sm.tile([HD, QT_W], F32, name="lb", tag="lb")
                    nc.gpsimd.partition_broadcast(lb[:], rr[0:1, :])
                    nc.vector.tensor_tensor(
                        cT[off : off + HD, t * QT_W : (t + 1) * QT_W],
                        ctx[HD:P, :],
                        lb[:],
                        mybir.AluOpType.mult,
                    )

                def outproj(t):
                    # each 384-wide f32 half gets its own 1-bank psum tile
                    for b in range(QT_W // P):
                        st = t * 4 + b
                        o_stage = sm.tile([P, D], BF16, name="o_stage", tag="ost")
                        for nch in range(2):
                            po = scp.tile([P, QT_W], F32, name="ctx", tag="ctx")
                            nc.tensor.matmul(
                                po[:, 0 : D // 2],
                                cT[:, st * P : (st + 1) * P],
                                wo_r[:, nch * (D // 2) : (nch + 1) * (D // 2)],
                                start=True,
                                stop=True,
                            )
                            nc.vector.tensor_copy(
                                o_stage[:, nch * (D // 2) : (nch + 1) * (D // 2)],
                                po[:, 0 : D // 2],
                            )
                        nc.sync.dma_start(out_d[st * P : (st + 1) * P, :], o_stage[:])

                for t in range(NQT):
                    attend(t, 0)
                    attend(t, 1)
                    # out-projection of the PREVIOUS tile: by now its
                    # normalization chain (recip/broadcast/mult) has long
                    # finished, so these matmuls never stall the PE queue
                    if t > 0:
                        outproj(t - 1)
                outproj(NQT - 1)

                if DEBUG:
                    nc.sync.dma_start(dbg_x_d[:], xT[:])
                    nc.sync.dma_start(dbg_q_d[:], qT[:])
                    nc.sync.dma_start(dbg_k0_d[:], k2[0][:])
                    nc.sync.dma_start(dbg_va_d[:], vA[:])
                    nc.sync.dma_start(dbg_ct_d[:], cT[:])

    nc.compile()
    return nc


def _host_inputs(x, W_query, W_key, W_value, W_out):
    mask = np.where(
        np.arange(P)[:, None] <= np.arange(P)[None, :], 0.0, -1e30
    ).astype(np.float32)
    ident = np.eye(P, dtype=ml_dtypes.bfloat16)
    xt = np.ascontiguousarray(x.T).astype(ml_dtypes.bfloat16)
    in_maps = []
    for core in range(8):
        ha, hb = SLOTS[core]
        sa, sb = SCALES[core]
        ca, cb = slice(ha * HD, (ha + 1) * HD), slice(hb * HD, (hb + 1) * HD)
        in_maps.append(
            {
                "xt": xt,
                "wq": np.ascontiguousarray(
                    np.concatenate([W_query[:, ca], W_query[:, cb]], axis=1)
                ).astype(ml_dtypes.bfloat16),
                "wk": np.ascontiguousarray(
                    np.concatenate([W_key[:, ca], W_key[:, cb]], axis=1)
                ).astype(ml_dtypes.bfloat16),
                "wv": np.ascontiguousarray(
                    np.concatenate([W_value[:, ca], W_value[:, cb]], axis=1)
                ).astype(ml_dtypes.bfloat16),
                "wo": np.ascontiguousarray(
                    np.concatenate([W_out[ca, :] * sa, W_out[cb, :] * sb], axis=0)
                ).astype(ml_dtypes.bfloat16),
                "mask": mask,
                "ident": ident,
            }
        )
    return in_maps


def run(x, W_query, W_key, W_value, W_out, b_out, trace=False):
    global _CACHED_NC
    if _CACHED_NC is None:
        _CACHED_NC = build_nc()
    nc = _CACHED_NC
    in_maps = _host_inputs(x, W_query, W_key, W_value, W_out)
    res = run_bass_kernel_spmd(nc, in_maps, core_ids=list(range(8)), trace=trace)
    out = np.zeros((S, D), dtype=np.float32)
    for core in range(8):
        out += res.results[core]["out"].astype(np.float32)
    out += b_out[None, :].astype(np.float32)
    return out, res


def kernel(x, W_query, W_key, W_value, W_out, b_out):
    x2 = np.asarray(x, dtype=np.float32).reshape(S, D)
    out, _ = run(
        x2,
        np.asarray(W_query, np.float32),
        np.asarray(W_key, np.float32),
        np.asarray(W_value, np.float32),
        np.asarray(W_out, np.float32),
        np.asarray(b_out, np.float32),
    )
    return out.reshape(1, S, D)
